# revision 24
# baseline (speedup 1.0000x reference)
"""Trainium2 Bass kernel for banded local attention (kernel_size=128).

Problem: x[4,4096,512]; q = x@Wq.T+bq, k = x@Wk.T+bk (H=512);
scores = q@k.T masked to |i-j|<128; softmax; out = attn @ x.

Algebraic restructure: softmax is shift-invariant per row, so terms of
q_i.k_j constant in j drop out:
    q_i . k_j  ~  (x_i (Wq^T Wk) + bq^T Wk) . x_j = t_i . x_j
with A = Wq^T @ Wk [D,D] and wbeta = Wk^T @ bq folded on the host.
This removes the whole k projection from the device: one projection
t = x@A + wbeta, then s = t @ x^T over a 384-wide sliding window,
softmax, out = p @ x.

Softmax path: negated row max over the RAW (unmasked) window (extra
terms only shift the max; the shift cancels in p/l), the band mask is
applied ADDITIVELY to the scores (s + 0/-60000) before the exp, so the
exp output is already-masked p AND its accum_out gives the softmax
denominator l for free. r = 1/l on DVE; the PSUM->SBUF evacuation of
the output doubles as the normalization multiply. Nothing ships to the
host except the finished o.

Sharding: 8 cores = 4 batches x 2 sequence halves (2048 queries each)
with 128-row key halos (2304 local rows; the global-edge halo rows are
zeroed ON DEVICE via memset, not DMAed). The h=1 half is passed
REVERSED so the padded region is always local rows [0,128) -> all 8
cores run the identical program (pure SPMD, no collectives). Host
un-reverses the h=1 outputs.

Schedule: ~16 junk matmuls on a resident tile spin the PE while the
first input DMAs stream, so the HAM clock is at full rate (2.4GHz)
before the first real matmul. xT arrives in 5 column pieces; the t
projection is emitted in chunks (all >=256 wide - fp32r matmuls run
4x slower below 256 free) interleaved with the 16 attention blocks, 4
blocks in flight (LAG=3). Mask + identity are generated on device
(iota/affine_select), keeping the startup-critical DMA stream to just
A + the first xT piece. The last four attention stages are
software-pipelined (transpose[b+1] overlaps out-matmul[b]) so the PE
never waits on a PSUM->SBUF copy in the drain.
PSUM banks: 2x[128,512] proj/out + 4x[128,384] scores +
2x[128,3,128] transpose = 8.
"""
import sys

if "/opt/trn_rl_repo" not in sys.path:
    sys.path.insert(0, "/opt/trn_rl_repo")

import numpy as np

B, S, D, H = 4, 4096, 512, 512
KS = 128
HALF = S // 2            # 2048 queries per core
HALO = KS                # 128
SK = HALF + 2 * HALO     # 2304 local key rows (incl. 128 zeroed on device)
SKD = SK - HALO          # 2176 rows actually DMAed
WIN = 3 * 128            # 384-wide key window per query block
NBLK = HALF // 128       # 16 query blocks
N_CORES = 8
DT = D // 128            # 4 contraction tiles
NEGBIG = -60000.0        # additive mask value (exp -> exact 0 in f32)
N_WARM = 18              # PE-warmup junk matmuls (HAM ramp during DMA fill)
N_WARM_GAP = 2           # extra warmups between tc0's DMA-paced groups
# xT column pieces in SBUF coords (col 0..127 is the memset zero halo)
XCH = [(128, 512), (512, 1024), (1024, 1536), (1536, 2048), (2048, 2304)]
# t-chunks as (tT col start, width); every width >=256 so fp32r matmuls
# run at full rate. chunk c's rhs (xT cols c0+128..c0+cw+128) lies in
# xT pieces <= c (except chunk 4 which also needs piece 4's start).
TCH = [(0, 384), (384, 512), (896, 512), (1408, 384), (1792, 256)]
# first block of each t-chunk: blocks [TBLK[c], TBLK[c+1]) need chunk c
TBLK = [0, 3, 7, 11, 14, 16]

_cached = {}


def _build_program():
    import concourse.bass as bass
    import concourse.tile as tile
    import concourse.mybir as mybir
    from concourse import bacc

    f32 = mybir.dt.float32
    f32r = mybir.dt.float32r
    bf16 = mybir.dt.bfloat16
    AF = mybir.ActivationFunctionType
    AX = mybir.AxisListType
    OP = mybir.AluOpType

    nc = bacc.Bacc("TRN2", target_bir_lowering=False, debug=False,
                   num_devices=N_CORES)

    A_d = nc.dram_tensor("A", [D, D], f32r, kind="ExternalInput").ap()
    wb_d = nc.dram_tensor("wb", [D, 1], f32, kind="ExternalInput").ap()
    xT_d = nc.dram_tensor("xT", [D, SKD], f32r, kind="ExternalInput").ap()
    xr_d = nc.dram_tensor("xr", [SKD, D], bf16, kind="ExternalInput").ap()
    o_d = nc.dram_tensor("o", [HALF, D], bf16, kind="ExternalOutput").ap()

    with tile.TileContext(nc) as tc:
        with (
            tc.tile_pool(name="big", bufs=1) as big,
            tc.tile_pool(name="pp", bufs=6) as pp,
            tc.tile_pool(name="psm", bufs=3) as psm,
            tc.tile_pool(name="ppt", bufs=4) as ppt,
            tc.tile_pool(name="po", bufs=4) as po,
            tc.tile_pool(name="stat", bufs=10) as stat,
            tc.tile_pool(name="psPO", bufs=2, space="PSUM") as psPO,
            tc.tile_pool(name="psS", bufs=4, space="PSUM") as psS,
            tc.tile_pool(name="psT", bufs=2, space="PSUM") as psT,
        ):
            # ---- resident tiles ----
            warm = big.tile([128, 512], bf16, tag="warm", name="warm")
            ident = big.tile([128, 128], bf16, tag="id", name="ident")
            mk = big.tile([128, 2, WIN], bf16, tag="mk", name="mk")
            wb = big.tile([128, DT], f32, tag="wb", name="wb")
            A_sb = big.tile([128, DT, D], f32r, tag="A", name="A")
            xT = big.tile([128, DT, SK], f32r, tag="xT", name="xT")
            xr = big.tile([128, SK // 128, D], bf16, tag="xr", name="xr")
            tT = big.tile([128, DT, HALF], f32r, tag="tT", name="tT")
            lcol = big.tile([128, NBLK], f32, tag="l", name="lcol")
            rcol = big.tile([128, NBLK], f32, tag="r", name="rcol")

            # ---- PE-warmup tile first (the junk matmuls depend on it)
            nc.vector.memset(warm, 1.0)

            # ---- PE warmup: junk matmuls with no input deps ramp the
            # HAM clock to 2.4GHz while A/xT stream in ----
            warm_n = [0]

            def emit_warm(n):
                for _ in range(n):
                    w = warm_n[0]
                    warm_n[0] += 1
                    wps = psPO.tile([128, 512], f32, tag="po",
                                    name=f"warm{w}")
                    nc.tensor.matmul(wps, lhsT=warm[:, 0:128], rhs=warm,
                                     start=True, stop=True)

            emit_warm(N_WARM)

            # ---- input DMAs. The DIRECT2D issue itself costs ~650ns
            # on the issuing sequencer, so the startup-critical set
            # (A + xT piece 0, per contraction tile) is issued from
            # FOUR sequencers in parallel; everything later streams
            # from the sync queue. ----
            xT_src = xT_d.rearrange("(t p) c -> p t c", t=DT)
            xr_src = xr_d.rearrange("(j p) d -> p j d", j=SKD // 128)

            def dma_crit(eng, dt_i):
                nc.__getattribute__(eng).dma_start(
                    A_sb[:, dt_i, :], A_d[dt_i * 128:(dt_i + 1) * 128, :])
                c0, c1 = XCH[0]
                nc.__getattribute__(eng).dma_start(
                    xT[:, dt_i, c0:c1],
                    xT_d[dt_i * 128:(dt_i + 1) * 128, c0 - HALO:c1 - HALO])

            def dma_xT_piece(c):
                c0, c1 = XCH[c]
                nc.sync.dma_start(xT[:, :, c0:c1],
                                  xT_src[:, :, c0 - HALO:c1 - HALO])

            def dma_xr_piece(j0, j1):
                # SBUF row-block j holds data rows (j-1) (row 0 = zeros)
                nc.sync.dma_start(xr[:, j0:j1, :], xr_src[:, j0 - 1:j1 - 1, :])

            dma_crit("sync", 0)
            nc.scalar.dma_start(
                wb[:, :], wb_d.rearrange("(t p) o -> p (t o)", t=DT))
            dma_crit("scalar", 2)
            dma_crit("gpsimd", 3)
            dma_crit("sync", 1)
            for dt_i in range(DT):
                c0, c1 = XCH[1]
                nc.sync.dma_start(
                    xT[:, dt_i, c0:c1],
                    xT_d[dt_i * 128:(dt_i + 1) * 128, c0 - HALO:c1 - HALO])
            dma_xr_piece(1, 6)
            dma_xT_piece(2)
            dma_xr_piece(6, 12)
            dma_xT_piece(3)
            dma_xr_piece(12, 18)
            dma_xT_piece(4)

            # ---- on-device constants (no DMA): zero halos, identity,
            # additive band masks — behind the critical DMA issues on
            # their sequencers, all needed only once blocks start ----
            nc.gpsimd.memset(xT[:, :, 0:HALO].bitcast(f32), 0.0)
            nc.gpsimd.memset(xr[:, 0, :], 0.0)
            # identity: zeros, fill 1.0 where c - r != 0 is false
            nc.gpsimd.memset(ident, 0.0)
            nc.gpsimd.affine_select(
                ident, ident, pattern=[[-1, 128]], base=0,
                channel_multiplier=1, compare_op=OP.not_equal, fill=1.0)
            # base mask: 0 where 1 <= c - r <= 255 else NEGBIG
            nc.gpsimd.memset(mk[:, 0, :], 0.0)
            nc.gpsimd.affine_select(
                mk[:, 0, :], mk[:, 0, :], pattern=[[1, WIN]], base=-1,
                channel_multiplier=-1, compare_op=OP.is_ge, fill=NEGBIG)
            nc.gpsimd.affine_select(
                mk[:, 0, :], mk[:, 0, :], pattern=[[-1, WIN]], base=255,
                channel_multiplier=1, compare_op=OP.is_ge, fill=NEGBIG)
            # edge mask (block 0): also kill the zero-padded cols < 128
            nc.gpsimd.tensor_copy(mk[:, 1, :], mk[:, 0, :])
            nc.gpsimd.affine_select(
                mk[:, 1, :], mk[:, 1, :], pattern=[[1, WIN]], base=-HALO,
                channel_multiplier=0, compare_op=OP.is_ge, fill=NEGBIG)

            # ---- emitters ----
            def emit_tchunk0():
                # first t-chunk, dt-major: each arriving A/xT per-dt
                # piece feeds 4 matmuls (one per ht) instead of 1, so
                # the PE overlaps the fill stream. Its four accumulation
                # groups borrow the score banks (same [128,384] f32
                # shape), idle until the first scores anyway.
                c0, cw = TCH[0]
                pss = [psS.tile([128, WIN], f32, tag="s", name=f"tc0p{h}")
                       for h in range(DT)]
                # dt order matches the multi-queue DMA arrival order;
                # junk warmups between the DMA-paced groups keep the
                # PE busy (and the HAM at full clock) through input
                # jitter — they target the idle proj banks, so they
                # run the moment the FIFO reaches them
                for k, dt_i in enumerate([0, 3, 2, 1]):
                    for ht in range(DT):
                        nc.tensor.matmul(
                            pss[ht],
                            lhsT=A_sb[:, dt_i, ht * 128:(ht + 1) * 128],
                            rhs=xT[:, dt_i, HALO + c0:HALO + c0 + cw],
                            start=(k == 0),
                            stop=(k == DT - 1),
                        )
                    if k < DT - 1:
                        emit_warm(N_WARM_GAP)
                for ht in range(DT):
                    if ht == 0:
                        nc.scalar.activation(
                            tT[:, ht, c0:c0 + cw], pss[ht],
                            AF.Identity, bias=wb[:, ht:ht + 1], scale=1.0)
                    else:
                        nc.vector.tensor_scalar_add(
                            tT[:, ht, c0:c0 + cw], pss[ht],
                            wb[:, ht:ht + 1])

            def emit_tchunk(c, h0, h1):
                # tT[:, ht, c0:c0+cw] = (x @ A + wbeta).T chunk, hts
                # [h0,h1) — chunks are emitted in halves between blocks
                # so the 2 proj PSUM banks never wait on their evacs,
                # which alternate scalar/DVE to split the queue load
                c0, cw = TCH[c]
                for ht in range(h0, h1):
                    ps = psPO.tile([128, 512], f32, tag="po")
                    for dt_i in range(DT):
                        nc.tensor.matmul(
                            ps[:, :cw],
                            lhsT=A_sb[:, dt_i, ht * 128:(ht + 1) * 128],
                            rhs=xT[:, dt_i, HALO + c0:HALO + c0 + cw],
                            start=(dt_i == 0),
                            stop=(dt_i == DT - 1),
                        )
                    if ht % 2 == 0:
                        nc.scalar.activation(
                            tT[:, ht, c0:c0 + cw], ps[:, :cw],
                            AF.Identity, bias=wb[:, ht:ht + 1], scale=1.0)
                    else:
                        nc.vector.tensor_scalar_add(
                            tT[:, ht, c0:c0 + cw], ps[:, :cw],
                            wb[:, ht:ht + 1])

            def emit_scores(b):
                j0 = b * 128
                s_ps = psS.tile([128, WIN], f32, tag="s")
                for ht in range(DT):
                    nc.tensor.matmul(
                        s_ps,
                        lhsT=tT[:, ht, j0:j0 + 128],
                        rhs=xT[:, ht, j0:j0 + WIN],
                        start=(ht == 0),
                        stop=(ht == DT - 1),
                    )
                negm = stat.tile([128, 1], f32, tag="negm")
                nc.vector.reduce_max(negm, s_ps, axis=AX.X, negate=True)
                # additive band mask into SBUF (with the max, the only
                # readers of the score bank, freeing it early). GpSimd
                # can't read PSUM, so this must live on DVE.
                s_m = psm.tile([128, WIN], f32, tag="sm")
                nc.vector.tensor_tensor(s_m, s_ps,
                                        mk[:, 1 if b == 0 else 0, :],
                                        op=OP.add)
                # exp of the masked scores IS the masked p, and its
                # accum_out is the softmax denominator l — for free
                p_sb = pp.tile([128, WIN], bf16, tag="p")
                nc.scalar.activation(p_sb, s_m, AF.Exp,
                                     bias=negm, scale=1.0,
                                     accum_out=lcol[:, b:b + 1])
                return p_sb

            opair = {}
            odict = {}

            def emit_o(b):
                # PSUM->SBUF evacuation doubles as the softmax
                # normalization (x 1/l), alternating DVE/ACT. The
                # reciprocal is emitted here (not at the exp) so it
                # never head-of-line-blocks the DVE queue on its exp.
                o_ps = odict.pop(b)
                r_b = rcol[:, b:b + 1]
                nc.vector.reciprocal(r_b, lcol[:, b:b + 1])
                if b % 2 == 0:
                    opair[0] = po.tile([128, 2, 512], bf16, tag="o",
                                       name=f"opair{b}")
                o_sb = opair[0]
                # o11 goes to DVE too: the drain-time ACT queue holds
                # the last exps + pT copies and must not grow. The very
                # last evacuation (o15) is on the critical tail — split
                # it across ACT and DVE so it takes ~half the time.
                if b == 15:
                    nc.scalar.mul(o_sb[:, 1, 0:256], o_ps[:, 0:256], r_b)
                    nc.vector.tensor_scalar_mul(
                        o_sb[:, 1, 256:512], o_ps[:, 256:512], r_b)
                elif b % 2 == 0 or b == 11:
                    nc.vector.tensor_scalar_mul(o_sb[:, b % 2, :], o_ps, r_b)
                else:
                    nc.scalar.mul(o_sb[:, 1, :], o_ps, r_b)
                if b % 2 == 1:
                    nc.sync.dma_start(
                        o_d[(b - 1) * 128:(b + 1) * 128, :]
                        .rearrange("(q p) d -> p q d", q=2),
                        o_sb)

            def emit_ta_t(b, pm_sb):
                # transpose p for the out matmul; copy engine rotates
                # so no single queue gates the PE
                pT_ps = psT.tile([128, 3, 128], bf16, tag="pT")
                for jt in range(3):
                    nc.tensor.transpose(
                        pT_ps[:, jt, :],
                        pm_sb[:, jt * 128:(jt + 1) * 128],
                        ident)
                pT_sb = ppt.tile([128, 3, 128], bf16, tag="pTs")
                if b >= 12:
                    # drain region: alternate DVE/ACT so neither queue
                    # serializes the back-to-back TAs
                    if b % 2 == 0:
                        nc.vector.tensor_copy(pT_sb, pT_ps)
                    else:
                        nc.scalar.copy(pT_sb, pT_ps)
                else:
                    nc.scalar.copy(pT_sb, pT_ps)
                return pT_sb

            def emit_ta_m(b, pT_sb, defer_o=False):
                o_ps = psPO.tile([128, 512], f32, tag="po")
                for jt in range(3):
                    nc.tensor.matmul(
                        o_ps,
                        lhsT=pT_sb[:, jt, :],
                        rhs=xr[:, b + jt, :],
                        start=(jt == 0),
                        stop=(jt == 2),
                    )
                odict[b] = o_ps
                if not defer_o:
                    emit_o(b)

            def emit_ta(b, pm_sb, defer_o=False):
                emit_ta_m(b, emit_ta_t(b, pm_sb), defer_o=defer_o)

            # ---- pipelined emission, 4 blocks in flight. t-chunks are
            # emitted as early as their xT piece allows (not just in
            # time) so their PSUM-evac copies land while the engine
            # queues are still shallow. The last attention stages are
            # deferred so the final scores run back-to-back, then
            # software-pipelined (transpose b+1 under out-matmul b) to
            # drain without PE stalls. ----
            LAG = 3
            # after block b -> emit (chunk, ht0, ht1): chunks go in
            # halves so the 2 proj PSUM banks recycle under the next
            # block's score matmuls
            tc_at = {1: (1, 0, 2), 2: (1, 2, 4), 3: (2, 0, 2),
                     4: (2, 2, 4), 5: (3, 0, 2), 6: (3, 2, 4),
                     7: (4, 0, 2), 8: (4, 2, 4)}
            pms = {}
            emit_tchunk0()
            for b in range(NBLK):
                pms[b] = emit_scores(b)
                # lag 3 through TA7, skip one, lag 4 in the tail: spaces
                # the last scores (avoids score-bank stalls) while still
                # leaving 4 deferred TAs after S15 to cover the drain
                ta = b - LAG if LAG <= b < 11 else (b - 4 if b >= 12 else None)
                if ta is not None:
                    emit_ta(ta, pms.pop(ta), defer_o=(ta >= 8))
                    if ta >= 10:
                        emit_o(ta - 2)
                if b in tc_at:
                    emit_tchunk(*tc_at[b])
            # drain: TAs 12-15 with transposes pipelined ahead of the
            # out matmuls (psT has 2 banks)
            pT12 = emit_ta_t(12, pms.pop(12))
            pT13 = emit_ta_t(13, pms.pop(13))
            emit_ta_m(12, pT12, defer_o=True)
            emit_o(10)
            pT14 = emit_ta_t(14, pms.pop(14))
            emit_ta_m(13, pT13, defer_o=True)
            emit_o(11)
            pT15 = emit_ta_t(15, pms.pop(15))
            emit_ta_m(14, pT14, defer_o=True)
            emit_o(12)
            emit_ta_m(15, pT15, defer_o=True)
            emit_o(13)
            emit_o(14)
            emit_o(15)

    nc.compile()
    return nc


def _get_program():
    if "nc" not in _cached:
        _cached["nc"] = _build_program()
    return _cached["nc"]


def kernel(x, Wq_w, Wq_b, Wk_w, Wk_b, _trace=False):
    import ml_dtypes
    from concourse.bass_utils import run_bass_kernel_spmd

    x = np.asarray(x, np.float32)
    Wq_w = np.asarray(Wq_w, np.float64)
    Wk_w = np.asarray(Wk_w, np.float64)
    Wq_b = np.asarray(Wq_b, np.float64)

    # fold both projections into one: t = x@A + wbeta, scores = t @ x^T
    A = np.ascontiguousarray((Wq_w.T @ Wk_w).astype(np.float32))
    wbeta = (Wk_w.T @ Wq_b).astype(np.float32).reshape(D, 1)

    nc = _get_program()

    in_maps = []
    for core in range(N_CORES):
        b, h = divmod(core, 2)
        if h == 0:
            x_half = x[b, 0:HALF + HALO]
        else:
            x_half = x[b, S - HALF - HALO:][::-1]
        in_maps.append({
            "A": A,
            "wb": wbeta,
            "xT": np.ascontiguousarray(x_half.T),
            "xr": x_half.astype(ml_dtypes.bfloat16),
        })

    res = run_bass_kernel_spmd(nc, in_maps, core_ids=list(range(N_CORES)),
                               trace=_trace)
    _cached["last_result"] = res

    y = np.zeros((B, S, D), np.float32)
    for core in range(N_CORES):
        b, h = divmod(core, 2)
        o = np.asarray(res.results[core]["o"], np.float32)
        if h == 0:
            y[b, :HALF] = o
        else:
            y[b, HALF:] = o[::-1]
    return y


# revision 26
# speedup vs baseline: 1.0233x; 1.0233x over previous
"""Trainium2 Bass kernel for banded local attention (kernel_size=128).

Problem: x[4,4096,512]; q = x@Wq.T+bq, k = x@Wk.T+bk (H=512);
scores = q@k.T masked to |i-j|<128; softmax; out = attn @ x.

Algebraic restructure: softmax is shift-invariant per row, so terms of
q_i.k_j constant in j drop out:
    q_i . k_j  ~  (x_i (Wq^T Wk) + bq^T Wk) . x_j = t_i . x_j
with A = Wq^T @ Wk [D,D] and wbeta = Wk^T @ bq folded on the host.
This removes the whole k projection from the device: one projection
t = x@A + wbeta, then s = t @ x^T over a 384-wide sliding window,
softmax, out = p @ x.

Softmax path: negated row max over the RAW (unmasked) window (extra
terms only shift the max; the shift cancels in p/l), the band mask is
applied ADDITIVELY to the scores (s + 0/-60000) before the exp, so the
exp output is already-masked p AND its accum_out gives the softmax
denominator l for free. r = 1/l on DVE; the PSUM->SBUF evacuation of
the output doubles as the normalization multiply. Nothing ships to the
host except the finished o.

Sharding: 8 cores = 4 batches x 2 sequence halves (2048 queries each)
with 128-row key halos (2304 local rows; the global-edge halo rows are
zeroed ON DEVICE via memset, not DMAed). The h=1 half is passed
REVERSED so the padded region is always local rows [0,128) -> all 8
cores run the identical program (pure SPMD, no collectives). Host
un-reverses the h=1 outputs.

Schedule: ~16 junk matmuls on a resident tile spin the PE while the
first input DMAs stream, so the HAM clock is at full rate (2.4GHz)
before the first real matmul. xT arrives in 5 column pieces; the t
projection is emitted in chunks (all >=256 wide - fp32r matmuls run
4x slower below 256 free) interleaved with the 16 attention blocks, 4
blocks in flight (LAG=3). Mask + identity are generated on device
(iota/affine_select), keeping the startup-critical DMA stream to just
A + the first xT piece. The last four attention stages are
software-pipelined (transpose[b+1] overlaps out-matmul[b]) so the PE
never waits on a PSUM->SBUF copy in the drain.
PSUM banks: 2x[128,512] proj/out + 4x[128,384] scores +
2x[128,3,128] transpose = 8.
"""
import sys

if "/opt/trn_rl_repo" not in sys.path:
    sys.path.insert(0, "/opt/trn_rl_repo")

import numpy as np

B, S, D, H = 4, 4096, 512, 512
KS = 128
HALF = S // 2            # 2048 queries per core
HALO = KS                # 128
SK = HALF + 2 * HALO     # 2304 local key rows (incl. 128 zeroed on device)
SKD = SK - HALO          # 2176 rows actually DMAed
WIN = 3 * 128            # 384-wide key window per query block
NBLK = HALF // 128       # 16 query blocks
N_CORES = 8
DT = D // 128            # 4 contraction tiles
NEGBIG = -60000.0        # additive mask value (exp -> exact 0 in f32)
N_WARM = 20              # PE-warmup junk matmuls (HAM ramp during DMA fill)
N_WARM_GAP = 4           # extra warmups between tc0's DMA-paced groups
# xT column pieces in SBUF coords (col 0..127 is the memset zero halo)
XCH = [(128, 512), (512, 1024), (1024, 1536), (1536, 2048), (2048, 2304)]
# t-chunks as (tT col start, width); every width >=256 so fp32r matmuls
# run at full rate. chunk c's rhs (xT cols c0+128..c0+cw+128) lies in
# xT pieces <= c (except chunk 4 which also needs piece 4's start).
TCH = [(0, 384), (384, 512), (896, 512), (1408, 384), (1792, 256)]
# first block of each t-chunk: blocks [TBLK[c], TBLK[c+1]) need chunk c
TBLK = [0, 3, 7, 11, 14, 16]

_cached = {}


def _build_program():
    import concourse.bass as bass
    import concourse.tile as tile
    import concourse.mybir as mybir
    from concourse import bacc

    f32 = mybir.dt.float32
    f32r = mybir.dt.float32r
    bf16 = mybir.dt.bfloat16
    AF = mybir.ActivationFunctionType
    AX = mybir.AxisListType
    OP = mybir.AluOpType

    nc = bacc.Bacc("TRN2", target_bir_lowering=False, debug=False,
                   num_devices=N_CORES)

    A_d = nc.dram_tensor("A", [D, D], f32r, kind="ExternalInput").ap()
    wb_d = nc.dram_tensor("wb", [D, 1], f32, kind="ExternalInput").ap()
    xT_d = nc.dram_tensor("xT", [D, SKD], f32r, kind="ExternalInput").ap()
    xr_d = nc.dram_tensor("xr", [SKD, D], bf16, kind="ExternalInput").ap()
    o_d = nc.dram_tensor("o", [HALF, D], bf16, kind="ExternalOutput").ap()

    with tile.TileContext(nc) as tc:
        with (
            tc.tile_pool(name="big", bufs=1) as big,
            tc.tile_pool(name="pp", bufs=6) as pp,
            tc.tile_pool(name="psm", bufs=3) as psm,
            tc.tile_pool(name="ppt", bufs=4) as ppt,
            tc.tile_pool(name="po", bufs=4) as po,
            tc.tile_pool(name="stat", bufs=10) as stat,
            tc.tile_pool(name="psPO", bufs=2, space="PSUM") as psPO,
            tc.tile_pool(name="psS", bufs=4, space="PSUM") as psS,
            tc.tile_pool(name="psT", bufs=2, space="PSUM") as psT,
        ):
            # ---- resident tiles ----
            warm = big.tile([128, 512], bf16, tag="warm", name="warm")
            ident = big.tile([128, 128], bf16, tag="id", name="ident")
            mk = big.tile([128, 2, WIN], bf16, tag="mk", name="mk")
            wb = big.tile([128, DT], f32, tag="wb", name="wb")
            A_sb = big.tile([128, DT, D], f32r, tag="A", name="A")
            xT = big.tile([128, DT, SK], f32r, tag="xT", name="xT")
            xr = big.tile([128, SK // 128, D], bf16, tag="xr", name="xr")
            tT = big.tile([128, DT, HALF], f32r, tag="tT", name="tT")
            lcol = big.tile([128, NBLK], f32, tag="l", name="lcol")
            rcol = big.tile([128, NBLK], f32, tag="r", name="rcol")

            # ---- PE-warmup tile first (the junk matmuls depend on it)
            nc.vector.memset(warm, 1.0)

            # ---- PE warmup: junk matmuls with no input deps ramp the
            # HAM clock to 2.4GHz while A/xT stream in ----
            warm_n = [0]

            def emit_warm(n):
                for _ in range(n):
                    w = warm_n[0]
                    warm_n[0] += 1
                    wps = psPO.tile([128, 512], f32, tag="po",
                                    name=f"warm{w}")
                    nc.tensor.matmul(wps, lhsT=warm[:, 0:128], rhs=warm,
                                     start=True, stop=True)

            emit_warm(N_WARM)

            # ---- input DMAs. The DIRECT2D issue itself costs ~650ns
            # on the issuing sequencer, so the startup-critical set
            # (A + xT piece 0, per contraction tile) is issued from
            # FOUR sequencers in parallel; everything later streams
            # from the sync queue. ----
            xT_src = xT_d.rearrange("(t p) c -> p t c", t=DT)
            xr_src = xr_d.rearrange("(j p) d -> p j d", j=SKD // 128)

            def dma_crit(eng, dt_i):
                nc.__getattribute__(eng).dma_start(
                    A_sb[:, dt_i, :], A_d[dt_i * 128:(dt_i + 1) * 128, :])
                c0, c1 = XCH[0]
                nc.__getattribute__(eng).dma_start(
                    xT[:, dt_i, c0:c1],
                    xT_d[dt_i * 128:(dt_i + 1) * 128, c0 - HALO:c1 - HALO])

            def dma_xT_piece(c):
                c0, c1 = XCH[c]
                nc.sync.dma_start(xT[:, :, c0:c1],
                                  xT_src[:, :, c0 - HALO:c1 - HALO])

            def dma_xr_piece(j0, j1):
                # SBUF row-block j holds data rows (j-1) (row 0 = zeros)
                nc.sync.dma_start(xr[:, j0:j1, :], xr_src[:, j0 - 1:j1 - 1, :])

            dma_crit("sync", 0)
            nc.scalar.dma_start(
                wb[:, :], wb_d.rearrange("(t p) o -> p (t o)", t=DT))
            dma_crit("scalar", 2)
            dma_crit("gpsimd", 3)
            dma_crit("sync", 1)
            for dt_i in range(DT):
                c0, c1 = XCH[1]
                nc.sync.dma_start(
                    xT[:, dt_i, c0:c1],
                    xT_d[dt_i * 128:(dt_i + 1) * 128, c0 - HALO:c1 - HALO])
            dma_xr_piece(1, 6)
            dma_xT_piece(2)
            dma_xr_piece(6, 12)
            dma_xT_piece(3)
            dma_xr_piece(12, 18)
            dma_xT_piece(4)

            # ---- on-device constants (no DMA): zero halos, identity,
            # additive band masks — behind the critical DMA issues on
            # their sequencers, all needed only once blocks start ----
            nc.gpsimd.memset(xT[:, :, 0:HALO].bitcast(f32), 0.0)
            nc.gpsimd.memset(xr[:, 0, :], 0.0)
            # identity: zeros, fill 1.0 where c - r != 0 is false
            nc.gpsimd.memset(ident, 0.0)
            nc.gpsimd.affine_select(
                ident, ident, pattern=[[-1, 128]], base=0,
                channel_multiplier=1, compare_op=OP.not_equal, fill=1.0)
            # base mask: 0 where 1 <= c - r <= 255 else NEGBIG
            nc.gpsimd.memset(mk[:, 0, :], 0.0)
            nc.gpsimd.affine_select(
                mk[:, 0, :], mk[:, 0, :], pattern=[[1, WIN]], base=-1,
                channel_multiplier=-1, compare_op=OP.is_ge, fill=NEGBIG)
            nc.gpsimd.affine_select(
                mk[:, 0, :], mk[:, 0, :], pattern=[[-1, WIN]], base=255,
                channel_multiplier=1, compare_op=OP.is_ge, fill=NEGBIG)
            # edge mask (block 0): also kill the zero-padded cols < 128
            nc.gpsimd.tensor_copy(mk[:, 1, :], mk[:, 0, :])
            nc.gpsimd.affine_select(
                mk[:, 1, :], mk[:, 1, :], pattern=[[1, WIN]], base=-HALO,
                channel_multiplier=0, compare_op=OP.is_ge, fill=NEGBIG)

            # ---- emitters ----
            def emit_tchunk0():
                # first t-chunk, dt-major: each arriving A/xT per-dt
                # piece feeds 4 matmuls (one per ht) instead of 1, so
                # the PE overlaps the fill stream. Its four accumulation
                # groups borrow the score banks (same [128,384] f32
                # shape), idle until the first scores anyway.
                c0, cw = TCH[0]
                pss = [psS.tile([128, WIN], f32, tag="s", name=f"tc0p{h}")
                       for h in range(DT)]
                # dt order matches the multi-queue DMA arrival order;
                # junk warmups between the DMA-paced groups keep the
                # PE busy (and the HAM at full clock) through input
                # jitter — they target the idle proj banks, so they
                # run the moment the FIFO reaches them
                for k, dt_i in enumerate([0, 3, 2, 1]):
                    for ht in range(DT):
                        nc.tensor.matmul(
                            pss[ht],
                            lhsT=A_sb[:, dt_i, ht * 128:(ht + 1) * 128],
                            rhs=xT[:, dt_i, HALO + c0:HALO + c0 + cw],
                            start=(k == 0),
                            stop=(k == DT - 1),
                        )
                    if k < DT - 1:
                        emit_warm(N_WARM_GAP)
                for ht in range(DT):
                    if ht == 0:
                        nc.scalar.activation(
                            tT[:, ht, c0:c0 + cw], pss[ht],
                            AF.Identity, bias=wb[:, ht:ht + 1], scale=1.0)
                    else:
                        nc.vector.tensor_scalar_add(
                            tT[:, ht, c0:c0 + cw], pss[ht],
                            wb[:, ht:ht + 1])

            def emit_tchunk(c, h0, h1):
                # tT[:, ht, c0:c0+cw] = (x @ A + wbeta).T chunk, hts
                # [h0,h1) — chunks are emitted in halves between blocks
                # so the 2 proj PSUM banks never wait on their evacs,
                # which alternate scalar/DVE to split the queue load
                c0, cw = TCH[c]
                for ht in range(h0, h1):
                    ps = psPO.tile([128, 512], f32, tag="po")
                    for dt_i in range(DT):
                        nc.tensor.matmul(
                            ps[:, :cw],
                            lhsT=A_sb[:, dt_i, ht * 128:(ht + 1) * 128],
                            rhs=xT[:, dt_i, HALO + c0:HALO + c0 + cw],
                            start=(dt_i == 0),
                            stop=(dt_i == DT - 1),
                        )
                    if ht % 2 == 0:
                        nc.scalar.activation(
                            tT[:, ht, c0:c0 + cw], ps[:, :cw],
                            AF.Identity, bias=wb[:, ht:ht + 1], scale=1.0)
                    else:
                        nc.vector.tensor_scalar_add(
                            tT[:, ht, c0:c0 + cw], ps[:, :cw],
                            wb[:, ht:ht + 1])

            def emit_scores(b):
                j0 = b * 128
                s_ps = psS.tile([128, WIN], f32, tag="s")
                for ht in range(DT):
                    nc.tensor.matmul(
                        s_ps,
                        lhsT=tT[:, ht, j0:j0 + 128],
                        rhs=xT[:, ht, j0:j0 + WIN],
                        start=(ht == 0),
                        stop=(ht == DT - 1),
                    )
                negm = stat.tile([128, 1], f32, tag="negm")
                nc.vector.reduce_max(negm, s_ps, axis=AX.X, negate=True)
                # additive band mask into SBUF (with the max, the only
                # readers of the score bank, freeing it early). GpSimd
                # can't read PSUM, so this must live on DVE.
                s_m = psm.tile([128, WIN], f32, tag="sm")
                nc.vector.tensor_tensor(s_m, s_ps,
                                        mk[:, 1 if b == 0 else 0, :],
                                        op=OP.add)
                # exp of the masked scores IS the masked p, and its
                # accum_out is the softmax denominator l — for free
                p_sb = pp.tile([128, WIN], bf16, tag="p")
                nc.scalar.activation(p_sb, s_m, AF.Exp,
                                     bias=negm, scale=1.0,
                                     accum_out=lcol[:, b:b + 1])
                return p_sb

            opair = {}
            odict = {}

            def emit_o(b):
                # PSUM->SBUF evacuation doubles as the softmax
                # normalization (x 1/l), alternating DVE/ACT. The
                # reciprocal is emitted here (not at the exp) so it
                # never head-of-line-blocks the DVE queue on its exp.
                o_ps = odict.pop(b)
                r_b = rcol[:, b:b + 1]
                nc.vector.reciprocal(r_b, lcol[:, b:b + 1])
                if b % 2 == 0:
                    opair[0] = po.tile([128, 2, 512], bf16, tag="o",
                                       name=f"opair{b}")
                o_sb = opair[0]
                # o11 and o15 go to DVE too: the drain-time ACT queue
                # holds the last exps + pT copies and must not grow
                if b % 2 == 0 or b == 11 or b == 15:
                    nc.vector.tensor_scalar_mul(o_sb[:, b % 2, :], o_ps, r_b)
                else:
                    nc.scalar.mul(o_sb[:, 1, :], o_ps, r_b)
                if b % 2 == 1:
                    nc.sync.dma_start(
                        o_d[(b - 1) * 128:(b + 1) * 128, :]
                        .rearrange("(q p) d -> p q d", q=2),
                        o_sb)

            def emit_ta_t(b, pm_sb):
                # transpose p for the out matmul; copy engine rotates
                # so no single queue gates the PE
                pT_ps = psT.tile([128, 3, 128], bf16, tag="pT")
                for jt in range(3):
                    nc.tensor.transpose(
                        pT_ps[:, jt, :],
                        pm_sb[:, jt * 128:(jt + 1) * 128],
                        ident)
                pT_sb = ppt.tile([128, 3, 128], bf16, tag="pTs")
                if b >= 12:
                    # drain region: alternate DVE/ACT so neither queue
                    # serializes the back-to-back TAs
                    if b % 2 == 0:
                        nc.vector.tensor_copy(pT_sb, pT_ps)
                    else:
                        nc.scalar.copy(pT_sb, pT_ps)
                else:
                    nc.scalar.copy(pT_sb, pT_ps)
                return pT_sb

            def emit_ta_m(b, pT_sb, defer_o=False):
                o_ps = psPO.tile([128, 512], f32, tag="po")
                for jt in range(3):
                    nc.tensor.matmul(
                        o_ps,
                        lhsT=pT_sb[:, jt, :],
                        rhs=xr[:, b + jt, :],
                        start=(jt == 0),
                        stop=(jt == 2),
                    )
                odict[b] = o_ps
                if not defer_o:
                    emit_o(b)

            def emit_ta(b, pm_sb, defer_o=False):
                emit_ta_m(b, emit_ta_t(b, pm_sb), defer_o=defer_o)

            # ---- pipelined emission, 4 blocks in flight. t-chunks are
            # emitted as early as their xT piece allows (not just in
            # time) so their PSUM-evac copies land while the engine
            # queues are still shallow. The last attention stages are
            # deferred so the final scores run back-to-back, then
            # software-pipelined (transpose b+1 under out-matmul b) to
            # drain without PE stalls. ----
            LAG = 3
            # after block b -> emit (chunk, ht0, ht1): chunks go in
            # halves so the 2 proj PSUM banks recycle under the next
            # block's score matmuls
            tc_at = {1: (1, 0, 2), 2: (1, 2, 4), 3: (2, 0, 2),
                     4: (2, 2, 4), 5: (3, 0, 2), 6: (3, 2, 4),
                     7: (4, 0, 2), 8: (4, 2, 4)}
            pms = {}
            emit_tchunk0()
            for b in range(NBLK):
                pms[b] = emit_scores(b)
                # lag 3 through TA7, skip one, lag 4 in the tail: spaces
                # the last scores (avoids score-bank stalls) while still
                # leaving 4 deferred TAs after S15 to cover the drain
                ta = b - LAG if LAG <= b < 11 else (b - 4 if b >= 12 else None)
                if ta is not None:
                    emit_ta(ta, pms.pop(ta), defer_o=(ta >= 8))
                    if ta >= 10:
                        emit_o(ta - 2)
                if b in tc_at:
                    emit_tchunk(*tc_at[b])
            # drain: TAs 12-15 with transposes pipelined ahead of the
            # out matmuls (psT has 2 banks)
            pT12 = emit_ta_t(12, pms.pop(12))
            pT13 = emit_ta_t(13, pms.pop(13))
            emit_ta_m(12, pT12, defer_o=True)
            emit_o(10)
            pT14 = emit_ta_t(14, pms.pop(14))
            emit_ta_m(13, pT13, defer_o=True)
            emit_o(11)
            pT15 = emit_ta_t(15, pms.pop(15))
            emit_ta_m(14, pT14, defer_o=True)
            emit_o(12)
            emit_ta_m(15, pT15, defer_o=True)
            emit_o(13)
            emit_o(14)
            emit_o(15)

    nc.compile()
    return nc


def _get_program():
    if "nc" not in _cached:
        _cached["nc"] = _build_program()
    return _cached["nc"]


def kernel(x, Wq_w, Wq_b, Wk_w, Wk_b, _trace=False):
    import ml_dtypes
    from concourse.bass_utils import run_bass_kernel_spmd

    x = np.asarray(x, np.float32)
    Wq_w = np.asarray(Wq_w, np.float64)
    Wk_w = np.asarray(Wk_w, np.float64)
    Wq_b = np.asarray(Wq_b, np.float64)

    # fold both projections into one: t = x@A + wbeta, scores = t @ x^T
    A = np.ascontiguousarray((Wq_w.T @ Wk_w).astype(np.float32))
    wbeta = (Wk_w.T @ Wq_b).astype(np.float32).reshape(D, 1)

    nc = _get_program()

    in_maps = []
    for core in range(N_CORES):
        b, h = divmod(core, 2)
        if h == 0:
            x_half = x[b, 0:HALF + HALO]
        else:
            x_half = x[b, S - HALF - HALO:][::-1]
        in_maps.append({
            "A": A,
            "wb": wbeta,
            "xT": np.ascontiguousarray(x_half.T),
            "xr": x_half.astype(ml_dtypes.bfloat16),
        })

    res = run_bass_kernel_spmd(nc, in_maps, core_ids=list(range(N_CORES)),
                               trace=_trace)
    _cached["last_result"] = res

    y = np.zeros((B, S, D), np.float32)
    for core in range(N_CORES):
        b, h = divmod(core, 2)
        o = np.asarray(res.results[core]["o"], np.float32)
        if h == 0:
            y[b, :HALF] = o
        else:
            y[b, HALF:] = o[::-1]
    return y
